# revision 11
# baseline (speedup 1.0000x reference)
"""Set-Transformer encoder (2x SAB sigmoid-attention + PMA) on 8 TRN2 cores.

Sharding: core c handles batch b=c//2, query-half hf=c%2 (1024 of 2048 rows).
All data flows feature-major ([D=128 partitions, tokens]); the host supplies
X pre-transposed so the kernel needs no on-device transposes.  Between SAB
layers each core pair AllGathers its half of the layer output (bf16).  The
PMA + final projection are computed redundantly by both cores of a pair.

Matmul operands are bf16 (1 cycle/row on PE); accumulation and the residual
spine stay fp32.  The per-head (dh=32) attention matmuls use 32x32
tile_position packing, 8 concurrent tiles per group.
"""
import numpy as np

import concourse.bass as bass
import concourse.tile as tile
from concourse import mybir
from concourse.bass_utils import run_bass_kernel_spmd

B, N, D, H, DH, DOUT = 4, 2048, 128, 4, 32, 256
NQ = N // 2          # queries per core
QCH = 512            # query chunk (matmul moving-dim)
NKT = N // 128       # 16 key tiles
SCALE = 1.0 / np.sqrt(np.float32(D))  # 1/sqrt(128) logit scale

fp32 = mybir.dt.float32
bf16 = mybir.dt.bfloat16
ALU = mybir.AluOpType
SIG = mybir.ActivationFunctionType.Sigmoid
PAIRS = [[0, 1], [2, 3], [4, 5], [6, 7]]
DEBUG_TAPS = False


def _fix_excess_waits(nc):
    """walrus accepts very few sync waits per instruction; hoist excess
    waits onto preceding same-engine NOPs (same stream => same semantics)."""
    for f in nc.m.functions:
        for bb in f.blocks:
            new_list = []
            for ins in bb.instructions:
                si = ins.sync_info
                cap = 2 if isinstance(ins, mybir.InstEventSemaphore) else 1
                if si is not None and len(si.on_wait) > cap:
                    waits = list(si.on_wait)
                    excess, kept = waits[:-cap], waits[-cap:]
                    for j, w in enumerate(excess):
                        nop = mybir.InstNoOp(
                            name=f"{ins.name}-presync{j}", ins=[], outs=[]
                        )
                        nop.engine = ins.engine
                        nop.sync_info = mybir.SyncInfo(on_wait=[w], on_update=[])
                        nc.register_instruction(nop)
                        new_list.append(nop)
                    ins.sync_info = mybir.SyncInfo(
                        on_wait=kept, on_update=list(si.on_update)
                    )
                new_list.append(ins)
            bb.instructions = new_list


def _bcast(ap, n):
    """[128,1] AP -> [128,n] free-dim broadcast."""
    return ap.to_broadcast([ap.shape[0], n])


def _load_weights(nc, sbuf, name, shapes):
    """Cast-DMA a dict of fp32 DRAM params into bf16/fp32 SBUF tiles."""
    tiles = {}
    for key, (shape, dt) in shapes.items():
        p = nc.declare_dram_parameter(key, shape, fp32, isOutput=False)
        t = sbuf.tile(shape, dt, tag=f"in_{key}")
        if dt == fp32:
            nc.sync.dma_start(out=t[:], in_=p[:])
        else:
            nc.gpsimd.dma_start(out=t[:], in_=p[:])
        tiles[key] = t
    return tiles


def _sab(nc, pools, XTfull, XTq, w, i, tagp):
    """One SAB layer. XTfull: [128,2048] bf16 (keys), XTq: [128,1024] bf16
    (this core's queries). Returns XhT_half [128,1024] bf16."""
    sbuf, sbufA, psL, psO, psP, psF = (
        pools["sbuf"], pools["sbufA"], pools["psL"], pools["psO"], pools["psP"],
        pools["psF"],
    )
    Wq, Wk, Wv, Wo = w[f"w{i}q"], w[f"w{i}k"], w[f"w{i}v"], w[f"w{i}o"]
    bq, bk, bvb, bo = w[f"b{i}q"], w[f"b{i}k"], w[f"b{i}v"], w[f"b{i}o"]

    # --- projections ---
    KT = sbuf.tile([128, N], bf16, tag=f"{tagp}KT")
    for c in range(4):
        ps = psP.tile([128, QCH], fp32, tag="proj")
        nc.tensor.matmul(ps[:], lhsT=Wk[:], rhs=XTfull[:, c * QCH:(c + 1) * QCH],
                         start=True, stop=True)
        nc.vector.tensor_tensor(
            out=KT[:, c * QCH:(c + 1) * QCH], in0=ps[:],
            in1=_bcast(bk[:, 0:1], QCH), op=ALU.add)

    QTf = sbuf.tile([128, NQ], fp32, tag=f"{tagp}QTf")
    QTb = sbuf.tile([128, NQ], bf16, tag=f"{tagp}QTb")
    for c in range(2):
        ps = psP.tile([128, QCH], fp32, tag="proj")
        nc.tensor.matmul(ps[:], lhsT=Wq[:], rhs=XTq[:, c * QCH:(c + 1) * QCH],
                         start=True, stop=True)
        nc.vector.tensor_tensor(
            out=QTf[:, c * QCH:(c + 1) * QCH], in0=ps[:],
            in1=_bcast(bq[:, 0:1], QCH), op=ALU.add)
        nc.vector.tensor_tensor(
            out=QTb[:, c * QCH:(c + 1) * QCH], in0=ps[:],
            in1=_bcast(bq[:, 0:1], QCH), op=ALU.add)

    # V token-major: V[p, 128*t + d] = (X @ Wv + bv)[128*t + p, d]
    V = sbuf.tile([128, N], bf16, tag=f"{tagp}V")
    for t in range(NKT):
        ps = psP.tile([128, 512], fp32, tag="proj")
        nc.tensor.matmul(ps[:, 0:128], lhsT=XTfull[:, t * 128:(t + 1) * 128],
                         rhs=Wv[:], start=True, stop=True)
        nc.vector.tensor_tensor(out=V[:, t * 128:(t + 1) * 128], in0=ps[:, 0:128],
                                in1=bvb[:], op=ALU.add)

    # --- attention ---
    XhT = sbuf.tile([128, NQ], bf16, tag=f"{tagp}XhT")
    OTf = sbuf.tile([128, NQ], fp32, tag=f"{tagp}OTf")
    OTb = sbuf.tile([128, NQ], bf16, tag=f"{tagp}OTb")
    for qc in range(2):
        qs = qc * QCH
        OTps = psO.tile([128, QCH], fp32, tag="OT")
        groups = [(kt, hp) for kt in range(NKT) for hp in range(2)]
        Ltiles = [None] * len(groups)

        def emit_qk(g):
            kt, hp = groups[g]
            L = psL.tile([128, 1024], fp32, tag="L")
            Ltiles[g] = L
            for h in (2 * hp, 2 * hp + 1):
                for j in range(4):
                    nc.tensor.matmul(
                        out=L[32 * j:32 * j + 32,
                              QCH * (h - 2 * hp):QCH * (h - 2 * hp) + QCH],
                        lhsT=KT[32 * h:32 * h + 32,
                                128 * kt + 32 * j:128 * kt + 32 * j + 32],
                        rhs=QTb[32 * h:32 * h + 32, qs:qs + QCH],
                        start=True, stop=True,
                        tile_position=(32 * h, 32 * j))
            return L

        emit_qk(0)
        for g in range(len(groups)):
            kt, hp = groups[g]
            if g + 1 < len(groups):
                emit_qk(g + 1)
            # sigmoid (with folded 1/sqrt(D) scale) PSUM -> SBUF bf16
            A = sbufA.tile([128, 1024], bf16, tag="A")
            nc.scalar.activation(A[:], Ltiles[g][:], SIG, scale=float(SCALE))
            Ltiles[g] = None
            # AV: accumulate O^T[32h:32h+32, q] over key tiles.
            # A holds all 128 keys of tile kt on partitions, so contract
            # the full K=128 with one col-banded matmul per head.
            for h in (2 * hp, 2 * hp + 1):
                nc.tensor.matmul(
                    out=OTps[32 * h:32 * h + 32, 0:QCH],
                    lhsT=V[:, 128 * kt + 32 * h:128 * kt + 32 * h + 32],
                    rhs=A[:, QCH * (h - 2 * hp):QCH * (h - 2 * hp) + QCH],
                    start=(kt == 0), stop=(kt == NKT - 1),
                    tile_position=(0, 32 * h),
                    skip_group_check=True)

        # O = Qp + A@V ; Xh = O + relu(O @ Wo + bo)
        nc.vector.tensor_tensor(out=OTf[:, qs:qs + QCH], in0=OTps[:],
                                in1=QTf[:, qs:qs + QCH], op=ALU.add)
        nc.vector.tensor_tensor(out=OTb[:, qs:qs + QCH], in0=OTps[:],
                                in1=QTf[:, qs:qs + QCH], op=ALU.add)
        FC = psF.tile([128, 512], fp32, tag="F")
        nc.tensor.matmul(FC[:], lhsT=Wo[:], rhs=OTb[:, qs:qs + QCH],
                         start=True, stop=True)
        R = sbuf.tile([128, QCH], fp32, tag="R")
        nc.vector.tensor_scalar(out=R[:], in0=FC[:], scalar1=bo[:, 0:1],
                                scalar2=0.0, op0=ALU.add, op1=ALU.max)
        nc.vector.tensor_tensor(out=XhT[:, qs:qs + QCH], in0=OTf[:, qs:qs + QCH],
                                in1=R[:], op=ALU.add)
    if DEBUG_TAPS:
        for nm, t in ((f"d{i}KT", KT), (f"d{i}QTb", QTb), (f"d{i}QTf", QTf),
                      (f"d{i}V", V), (f"d{i}OTf", OTf), (f"d{i}XhT", XhT)):
            dd = nc.declare_dram_parameter(nm, list(t[:].shape), fp32,
                                           isOutput=True)
            nc.gpsimd.dma_start(out=dd[:], in_=t[:])
    return XhT


def _allgather_half(nc, pools, XhT, tagp):
    """Pair-AllGather [128,1024] bf16 halves -> [128,2048] bf16 full."""
    dram = pools["dram"]
    sbuf = pools["sbuf"]
    cc_in = dram.tile([128, NQ], bf16, tag=f"{tagp}cci")
    nc.sync.dma_start(out=cc_in[:], in_=XhT[:])
    cc_out = dram.tile([256, NQ], bf16, tag=f"{tagp}cco")
    nc.gpsimd.collective_compute(
        "AllGather", ALU.bypass, replica_groups=PAIRS,
        ins=[cc_in[:]], outs=[cc_out[:]])
    XT = sbuf.tile([128, N], bf16, tag=f"{tagp}XTn")
    nc.sync.dma_start(out=XT[:, 0:NQ], in_=cc_out[0:128, :])
    nc.sync.dma_start(out=XT[:, NQ:N], in_=cc_out[128:256, :])
    return XT


def _pma(nc, pools, XTfull, w, extras):
    """PMA (1 seed) + final projection -> out [1, 256] fp32 in SBUF."""
    sbuf, psP, psF = pools["sbuf"], pools["psP"], pools["psF"]
    i = 2
    Wq, Wk, Wv, Wo = w[f"w{i}q"], w[f"w{i}k"], w[f"w{i}v"], w[f"w{i}o"]
    bq, bk, bvb, bo = w[f"b{i}q"], w[f"b{i}k"], w[f"b{i}v"], w[f"b{i}o"]
    ST, hmask, pW, pb = extras["st"], extras["hmask"], extras["pw"], extras["pb"]

    # Q_pma^T [128,1] = Wq.T @ S^T + bq
    psq = psP.tile([128, 512], fp32, tag="proj")
    nc.tensor.matmul(psq[:, 0:1], lhsT=Wq[:], rhs=ST[:, 0:1], start=True, stop=True)
    QpTf = sbuf.tile([128, 1], fp32, tag="QpTf")
    QpTb = sbuf.tile([128, 1], bf16, tag="QpTb")
    nc.vector.tensor_tensor(out=QpTf[:], in0=psq[:, 0:1], in1=bq[:, 0:1], op=ALU.add)
    nc.vector.tensor_copy(QpTb[:], QpTf[:])
    # Block-diagonal Qhat[d, h] = Qp^T[d] * (d//32 == h)
    Qhat = sbuf.tile([128, H], bf16, tag="Qhat")
    nc.vector.tensor_tensor(out=Qhat[:], in0=_bcast(QpTb[:, 0:1], H),
                            in1=hmask[:], op=ALU.mult)

    KT = sbuf.tile([128, N], bf16, tag="pKT")
    for c in range(4):
        ps = psP.tile([128, QCH], fp32, tag="proj")
        nc.tensor.matmul(ps[:], lhsT=Wk[:], rhs=XTfull[:, c * QCH:(c + 1) * QCH],
                         start=True, stop=True)
        nc.vector.tensor_tensor(out=KT[:, c * QCH:(c + 1) * QCH], in0=ps[:],
                                in1=_bcast(bk[:, 0:1], QCH), op=ALU.add)
    V = sbuf.tile([128, N], bf16, tag="pV")
    for t in range(NKT):
        ps = psP.tile([128, 512], fp32, tag="proj")
        nc.tensor.matmul(ps[:, 0:128], lhsT=XTfull[:, t * 128:(t + 1) * 128],
                         rhs=Wv[:], start=True, stop=True)
        nc.vector.tensor_tensor(out=V[:, t * 128:(t + 1) * 128], in0=ps[:, 0:128],
                                in1=bvb[:], op=ALU.add)

    # logits L[key, h] via block-diagonal Qhat (cross-head terms hit zeros)
    Lp_t = psF.tile([128, 512], fp32, tag="F")
    Lp = Lp_t[:, 0:4 * NKT]
    for t in range(NKT):
        nc.tensor.matmul(Lp[:, 4 * t:4 * t + 4],
                         lhsT=KT[:, t * 128:(t + 1) * 128], rhs=Qhat[:],
                         start=True, stop=True)
    Ap = sbuf.tile([128, 4 * NKT], bf16, tag="Ap")
    nc.scalar.activation(Ap[:], Lp[:], SIG, scale=float(SCALE))

    # O[1, d] = sum_m A_h(d)[m] V[m, d]
    Ops_t = psF.tile([128, 512], fp32, tag="F")
    Ops = Ops_t[0:1, 0:128]
    for h in range(H):
        for t in range(NKT):
            nc.tensor.matmul(
                Ops[0:1, 32 * h:32 * h + 32],
                lhsT=Ap[:, 4 * t + h:4 * t + h + 1],
                rhs=V[:, 128 * t + 32 * h:128 * t + 32 * h + 32],
                start=(t == 0), stop=(t == NKT - 1), skip_group_check=True)
    Ob = sbuf.tile([1, 128], bf16, tag="Ob")
    nc.vector.tensor_copy(Ob[:], Ops[:])
    # transpose O -> O^T via K=1 matmul with ones
    ones = sbuf.tile([1, 1], bf16, tag="ones")
    nc.vector.memset(ones[:], 1.0)
    OpTps_t = psF.tile([128, 512], fp32, tag="F")
    OpTps = OpTps_t[:, 0:1]
    nc.tensor.matmul(OpTps[:], lhsT=Ob[:], rhs=ones[:], start=True, stop=True)
    OpTf = sbuf.tile([128, 1], fp32, tag="OpTf")
    OpTb = sbuf.tile([128, 1], bf16, tag="OpTb")
    nc.vector.tensor_tensor(out=OpTf[:], in0=OpTps[:], in1=QpTf[:], op=ALU.add)
    nc.vector.tensor_tensor(out=OpTb[:], in0=OpTps[:], in1=QpTf[:], op=ALU.add)
    # fc_o + relu + residual
    FC2_t = psF.tile([128, 512], fp32, tag="F")
    FC2 = FC2_t[:, 0:1]
    nc.tensor.matmul(FC2[:], lhsT=Wo[:], rhs=OpTb[:], start=True, stop=True)
    R2 = sbuf.tile([128, 1], fp32, tag="R2")
    nc.vector.tensor_scalar(out=R2[:], in0=FC2[:], scalar1=bo[:, 0:1],
                            scalar2=0.0, op0=ALU.add, op1=ALU.max)
    XpTb = sbuf.tile([128, 1], bf16, tag="XpTb")
    nc.vector.tensor_tensor(out=XpTb[:], in0=OpTf[:], in1=R2[:], op=ALU.add)
    # final [1,256] = P @ pW + pb
    OUTps_t = psF.tile([128, 512], fp32, tag="F")
    OUTps = OUTps_t[0:1, 0:DOUT]
    nc.tensor.matmul(OUTps[:], lhsT=XpTb[:], rhs=pW[:], start=True, stop=True)
    out_sb = sbuf.tile([1, DOUT], fp32, tag="out_sb")
    nc.vector.tensor_tensor(out=out_sb[:], in0=OUTps[:], in1=pb[:], op=ALU.add)
    if DEBUG_TAPS:
        for nm, t in (("dpQpTf", QpTf), ("dpQhat", Qhat), ("dpKT", KT),
                      ("dpV", V), ("dpAp", Ap), ("dpOpTf", OpTf),
                      ("dpXpTb", XpTb)):
            dd = nc.declare_dram_parameter(nm, list(t[:].shape), fp32,
                                           isOutput=True)
            nc.gpsimd.dma_start(out=dd[:], in_=t[:])
    return out_sb


def build_program():
    nc = bass.Bass(num_devices=8)
    xt = nc.declare_dram_parameter("xt", [128, N], fp32, isOutput=False)
    xtq = nc.declare_dram_parameter("xtq", [128, NQ], fp32, isOutput=False)
    out_d = nc.declare_dram_parameter("out", [1, DOUT], fp32, isOutput=True)

    wshapes = {}
    for i in range(3):
        for k in ("q", "k", "v", "o"):
            wshapes[f"w{i}{k}"] = ([128, 128], bf16)
        wshapes[f"b{i}q"] = ([128, 1], fp32)
        wshapes[f"b{i}k"] = ([128, 1], fp32)
        wshapes[f"b{i}v"] = ([128, 128], fp32)  # pre-broadcast across partitions
        wshapes[f"b{i}o"] = ([128, 1], fp32)
    eshapes = {
        "st": ([128, 1], bf16),
        "hmask": ([128, H], bf16),
        "pw": ([128, DOUT], bf16),
        "pb": ([1, DOUT], fp32),
    }

    with tile.TileContext(nc) as tc:
        with (
            tc.tile_pool(name="sbuf", bufs=1) as sbuf,
            tc.tile_pool(name="sbufA", bufs=3) as sbufA,
            tc.tile_pool(name="psL", bufs=2, space="PSUM") as psL,
            tc.tile_pool(name="psO", bufs=1, space="PSUM") as psO,
            tc.tile_pool(name="psP", bufs=2, space="PSUM") as psP,
            tc.tile_pool(name="psF", bufs=1, space="PSUM") as psF,
            tc.tile_pool(name="dram", bufs=1, space="DRAM") as dram,
        ):
            pools = {"sbuf": sbuf, "sbufA": sbufA, "psL": psL, "psO": psO,
                     "psP": psP, "psF": psF, "dram": dram}

            w = _load_weights(nc, sbuf, "w", wshapes)
            extras = _load_weights(nc, sbuf, "e", eshapes)
            XT0 = sbuf.tile([128, N], bf16, tag="XT0")
            nc.gpsimd.dma_start(out=XT0[:], in_=xt[:])
            XTq0 = sbuf.tile([128, NQ], bf16, tag="XTq0")
            nc.gpsimd.dma_start(out=XTq0[:], in_=xtq[:])

            Xh0 = _sab(nc, pools, XT0, XTq0, w, 0, "s0")
            XT1 = _allgather_half(nc, pools, Xh0, "g0")
            Xh1 = _sab(nc, pools, XT1, Xh0, w, 1, "s1")
            XT2 = _allgather_half(nc, pools, Xh1, "g1")
            out_sb = _pma(nc, pools, XT2, w, extras)
            nc.sync.dma_start(out=out_d[:], in_=out_sb[:])

    _fix_excess_waits(nc)
    return nc


_CACHE = {}


def _inputs_for_core(inputs, c):
    b, hf = c // 2, c % 2
    X = np.asarray(inputs["X"], dtype=np.float32)
    XT = np.ascontiguousarray(X[b].T)
    m = {
        "xt": XT,
        "xtq": np.ascontiguousarray(XT[:, hf * NQ:(hf + 1) * NQ]),
        "st": np.ascontiguousarray(np.asarray(inputs["S"], np.float32).reshape(D, 1)),
        "hmask": (np.arange(128)[:, None] // 32 == np.arange(H)[None, :]
                  ).astype(np.float32),
        "pw": np.ascontiguousarray(np.asarray(inputs["pW"], np.float32)),
        "pb": np.asarray(inputs["pb"], np.float32).reshape(1, DOUT),
    }
    for i in range(3):
        for k in ("q", "k", "v", "o"):
            m[f"w{i}{k}"] = np.ascontiguousarray(
                np.asarray(inputs[f"m{i}_W{k}"], np.float32))
        m[f"b{i}q"] = np.asarray(inputs[f"m{i}_bq"], np.float32).reshape(128, 1)
        m[f"b{i}k"] = np.asarray(inputs[f"m{i}_bk"], np.float32).reshape(128, 1)
        m[f"b{i}v"] = np.tile(
            np.asarray(inputs[f"m{i}_bv"], np.float32)[None, :], (128, 1))
        m[f"b{i}o"] = np.asarray(inputs[f"m{i}_bo"], np.float32).reshape(128, 1)
    return m


def kernel(**inputs) -> np.ndarray:
    if "nc" not in _CACHE:
        _CACHE["nc"] = build_program()
    nc = _CACHE["nc"]
    in_maps = [_inputs_for_core(inputs, c) for c in range(8)]
    res = run_bass_kernel_spmd(nc, in_maps, list(range(8)))
    out = np.stack([res.results[2 * b]["out"] for b in range(B)], axis=0)
    return out.astype(np.float32)  # [B, 1, DOUT]


# revision 13
# speedup vs baseline: 1.0579x; 1.0579x over previous
"""Set-Transformer encoder (2x SAB sigmoid-attention + PMA) on 8 TRN2 cores.

Sharding: core c handles batch b=c//2, query-half hf=c%2 (1024 of 2048 rows).
All data flows feature-major ([D=128 partitions, tokens]); the host supplies
X pre-transposed and pre-cast to bf16.  Between SAB layers each core pair
AllGathers its half of the layer output in two query-chunks, launched as
soon as each chunk is ready so the exchange hides under the remaining
attention work; the next layer processes the keys covered by the first
chunk before the second arrives (attention is permutation-invariant over
keys).  The PMA + final projection are computed redundantly by both cores
of a pair.

Matmul operands are bf16 (1 cycle/row on PE); accumulation and the residual
spine stay fp32.  The per-head (dh=32) QK matmuls use 32x32 tile_position
packing (8 concurrent tiles per 128-key group); AV contracts the full 128
keys with col-banded (M=32) matmuls accumulating O^T in place.
"""
import numpy as np
import ml_dtypes

import concourse.bass as bass
import concourse.tile as tile
from concourse import mybir
from concourse.bass_utils import run_bass_kernel_spmd

B, N, D, H, DH, DOUT = 4, 2048, 128, 4, 32, 256
NQ = N // 2          # queries per core
QCH = 512            # query chunk (matmul moving-dim)
NKT = N // 128       # 16 key tiles
SCALE = 1.0 / np.sqrt(np.float32(D))  # 1/sqrt(128) logit scale

fp32 = mybir.dt.float32
bf16 = mybir.dt.bfloat16
ALU = mybir.AluOpType
SIG = mybir.ActivationFunctionType.Sigmoid
PAIRS = [[0, 1], [2, 3], [4, 5], [6, 7]]
DEBUG_TAPS = False

# key-tile processing order when keys arrive via 2-chunk AllGather:
# AG chunk a carries each core's queries [0:512) -> global keys
# [0:512) u [1024:1536) = key tiles 0-3 and 8-11.
KT_ORDER_AG = [0, 1, 2, 3, 8, 9, 10, 11, 4, 5, 6, 7, 12, 13, 14, 15]
KCH_ORDER_AG = [0, 2, 1, 3]          # 512-col projection chunk order


def _fix_excess_waits(nc):
    """walrus accepts very few sync waits per instruction; hoist excess
    waits onto preceding same-engine NOPs (same stream => same semantics)."""
    for f in nc.m.functions:
        for bb in f.blocks:
            new_list = []
            for ins in bb.instructions:
                si = ins.sync_info
                cap = 2 if isinstance(ins, mybir.InstEventSemaphore) else 1
                if si is not None and len(si.on_wait) > cap:
                    waits = list(si.on_wait)
                    excess, kept = waits[:-cap], waits[-cap:]
                    for j, w in enumerate(excess):
                        nop = mybir.InstNoOp(
                            name=f"{ins.name}-presync{j}", ins=[], outs=[]
                        )
                        nop.engine = ins.engine
                        nop.sync_info = mybir.SyncInfo(on_wait=[w], on_update=[])
                        nc.register_instruction(nop)
                        new_list.append(nop)
                    ins.sync_info = mybir.SyncInfo(
                        on_wait=kept, on_update=list(si.on_update)
                    )
                new_list.append(ins)
            bb.instructions = new_list


def _bcast(ap, n):
    return ap.to_broadcast([ap.shape[0], n])


def _load_weights(nc, sbuf, shapes):
    tiles = {}
    for key, (shape, dt) in shapes.items():
        p = nc.declare_dram_parameter(key, shape, dt if dt == bf16 else fp32,
                                      isOutput=False)
        t = sbuf.tile(shape, dt, tag=f"in_{key}")
        if dt == bf16:
            nc.sync.dma_start(out=t[:], in_=p[:])
        else:
            nc.sync.dma_start(out=t[:], in_=p[:])
        tiles[key] = t
    return tiles


def _projections(nc, pools, XTfull, XTq, w, i, tagp, kch_order, kt_order):
    """K^T / Q^T / V projections for one MAB layer."""
    sbuf, psP = pools["sbuf"], pools["psP"]
    Wq, Wk, Wv = w[f"w{i}q"], w[f"w{i}k"], w[f"w{i}v"]
    bq, bk, bvb = w[f"b{i}q"], w[f"b{i}k"], w[f"b{i}v"]

    QTf = sbuf.tile([128, NQ], fp32, tag=f"{tagp}QTf")
    QTb = sbuf.tile([128, NQ], bf16, tag=f"{tagp}QTb")
    for c in range(2):
        ps = psP.tile([128, QCH], fp32, tag="proj")
        nc.tensor.matmul(ps[:], lhsT=Wq[:], rhs=XTq[:, c * QCH:(c + 1) * QCH],
                         start=True, stop=True)
        nc.vector.tensor_tensor(
            out=QTf[:, c * QCH:(c + 1) * QCH], in0=ps[:],
            in1=_bcast(bq[:, 0:1], QCH), op=ALU.add)
        nc.vector.tensor_tensor(
            out=QTb[:, c * QCH:(c + 1) * QCH], in0=ps[:],
            in1=_bcast(bq[:, 0:1], QCH), op=ALU.add)

    KT = sbuf.tile([128, N], bf16, tag=f"{tagp}KT")
    for c in kch_order:
        ps = psP.tile([128, QCH], fp32, tag="proj")
        nc.tensor.matmul(ps[:], lhsT=Wk[:], rhs=XTfull[:, c * QCH:(c + 1) * QCH],
                         start=True, stop=True)
        nc.vector.tensor_tensor(
            out=KT[:, c * QCH:(c + 1) * QCH], in0=ps[:],
            in1=_bcast(bk[:, 0:1], QCH), op=ALU.add)

    # V token-major: V[p, 128*t + d] = (X @ Wv + bv)[128*t + p, d]
    V = sbuf.tile([128, N], bf16, tag=f"{tagp}V")
    for t in kt_order:
        ps = psP.tile([128, QCH], fp32, tag="proj")
        nc.tensor.matmul(ps[:, 0:128], lhsT=XTfull[:, t * 128:(t + 1) * 128],
                         rhs=Wv[:], start=True, stop=True)
        nc.vector.tensor_tensor(out=V[:, t * 128:(t + 1) * 128], in0=ps[:, 0:128],
                                in1=bvb[:], op=ALU.add)
    return KT, QTf, QTb, V


def _sab(nc, pools, XTfull, XTq, w, i, tagp, kch_order, kt_order, emit_ag):
    """One SAB layer; returns (XhT_half, XTnext or None)."""
    sbuf, sbufA, psL, psO, psF, dram = (
        pools["sbuf"], pools["sbufA"], pools["psL"], pools["psO"], pools["psF"],
        pools["dram"],
    )
    Wo, bo = w[f"w{i}o"], w[f"b{i}o"]
    KT, QTf, QTb, V = _projections(nc, pools, XTfull, XTq, w, i, tagp,
                                   kch_order, kt_order)

    XhT = sbuf.tile([128, NQ], bf16, tag=f"{tagp}XhT")
    OTf = sbuf.tile([128, NQ], fp32, tag=f"{tagp}OTf")
    OTb = sbuf.tile([128, NQ], bf16, tag=f"{tagp}OTb")
    XTnext = None
    if emit_ag:
        XTnext = sbuf.tile([128, N], bf16, tag=f"{tagp}XTn")

    for qc in range(2):
        qs = qc * QCH
        OTps = psO.tile([128, QCH], fp32, tag="OT")
        groups = [(kt, hp) for kt in kt_order for hp in range(2)]
        Ltiles = {}

        def emit_qk(g):
            kt, hp = groups[g]
            L = psL.tile([128, 1024], fp32, tag="L")
            Ltiles[g] = L
            for h in (2 * hp, 2 * hp + 1):
                for j in range(4):
                    nc.tensor.matmul(
                        out=L[32 * j:32 * j + 32,
                              QCH * (h - 2 * hp):QCH * (h - 2 * hp) + QCH],
                        lhsT=KT[32 * h:32 * h + 32,
                                128 * kt + 32 * j:128 * kt + 32 * j + 32],
                        rhs=QTb[32 * h:32 * h + 32, qs:qs + QCH],
                        start=True, stop=True,
                        tile_position=(32 * h, 32 * j))

        emit_qk(0)
        ng = len(groups)
        for g in range(ng):
            kt, hp = groups[g]
            if g + 1 < ng:
                emit_qk(g + 1)
            A = sbufA.tile([128, 1024], bf16, tag="A")
            nc.scalar.activation(A[:], Ltiles.pop(g)[:], SIG, scale=float(SCALE))
            # AV: A holds the full 128 keys of tile kt on partitions;
            # contract K=128 with one col-banded matmul per head.
            for h in (2 * hp, 2 * hp + 1):
                nc.tensor.matmul(
                    out=OTps[32 * h:32 * h + 32, 0:QCH],
                    lhsT=V[:, 128 * kt + 32 * h:128 * kt + 32 * h + 32],
                    rhs=A[:, QCH * (h - 2 * hp):QCH * (h - 2 * hp) + QCH],
                    start=(g // 2 == 0), stop=(g // 2 == NKT - 1),
                    tile_position=(0, 32 * h),
                    skip_group_check=True)

        # O = Qp + A@V ; Xh = O + relu(O @ Wo + bo)
        nc.vector.tensor_tensor(out=OTf[:, qs:qs + QCH], in0=OTps[:],
                                in1=QTf[:, qs:qs + QCH], op=ALU.add)
        nc.vector.tensor_tensor(out=OTb[:, qs:qs + QCH], in0=OTps[:],
                                in1=QTf[:, qs:qs + QCH], op=ALU.add)
        FC = psF.tile([128, QCH], fp32, tag="F")
        nc.tensor.matmul(FC[:], lhsT=Wo[:], rhs=OTb[:, qs:qs + QCH],
                         start=True, stop=True)
        R = sbuf.tile([128, QCH], fp32, tag="R")
        nc.vector.tensor_scalar(out=R[:], in0=FC[:], scalar1=bo[:, 0:1],
                                scalar2=0.0, op0=ALU.add, op1=ALU.max)
        nc.vector.tensor_tensor(out=XhT[:, qs:qs + QCH], in0=OTf[:, qs:qs + QCH],
                                in1=R[:], op=ALU.add)

        if emit_ag:
            # exchange this query chunk with the pair core right away
            cc_in = dram.tile([128, QCH], bf16, tag=f"{tagp}cci{qc}")
            nc.sync.dma_start(out=cc_in[:], in_=XhT[:, qs:qs + QCH])
            cc_out = dram.tile([256, QCH], bf16, tag=f"{tagp}cco{qc}")
            nc.gpsimd.collective_compute(
                "AllGather", ALU.bypass, replica_groups=PAIRS,
                ins=[cc_in[:]], outs=[cc_out[:]])
            # global columns: rank0 rows -> [qs:qs+512), rank1 -> [1024+qs:...)
            nc.sync.dma_start(out=XTnext[:, qs:qs + QCH], in_=cc_out[0:128, :])
            nc.sync.dma_start(out=XTnext[:, NQ + qs:NQ + qs + QCH],
                              in_=cc_out[128:256, :])

    if DEBUG_TAPS:
        for nm, t in ((f"d{i}KT", KT), (f"d{i}QTb", QTb), (f"d{i}QTf", QTf),
                      (f"d{i}V", V), (f"d{i}OTf", OTf), (f"d{i}XhT", XhT)):
            dd = nc.declare_dram_parameter(nm, list(t[:].shape), fp32,
                                           isOutput=True)
            nc.gpsimd.dma_start(out=dd[:], in_=t[:])
    return XhT, XTnext


def _pma_q(nc, pools, w, extras):
    """PMA seed query (depends only on S + mab2 weights) - emitted early."""
    sbuf, psP = pools["sbuf"], pools["psP"]
    Wq, bq = w["w2q"], w["b2q"]
    ST, hmask = extras["st"], extras["hmask"]
    psq = psP.tile([128, QCH], fp32, tag="proj")
    nc.tensor.matmul(psq[:, 0:1], lhsT=Wq[:], rhs=ST[:, 0:1], start=True,
                     stop=True)
    QpTf = sbuf.tile([128, 1], fp32, tag="QpTf")
    QpTb = sbuf.tile([128, 1], bf16, tag="QpTb")
    nc.vector.tensor_tensor(out=QpTf[:], in0=psq[:, 0:1], in1=bq[:, 0:1],
                            op=ALU.add)
    nc.vector.tensor_copy(QpTb[:], QpTf[:])
    # Block-diagonal Qhat[d, h] = Qp^T[d] * (d//32 == h)
    Qhat = sbuf.tile([128, H], bf16, tag="Qhat")
    nc.vector.tensor_tensor(out=Qhat[:], in0=_bcast(QpTb[:, 0:1], H),
                            in1=hmask[:], op=ALU.mult)
    return QpTf, Qhat


def _pma(nc, pools, XTfull, w, extras, QpTf, Qhat, kch_order, kt_order):
    """PMA (1 seed) + final projection -> out [1, 256] fp32 in SBUF."""
    sbuf, psP, psF = pools["sbuf"], pools["psP"], pools["psF"]
    Wk, Wv, Wo = w["w2k"], w["w2v"], w["w2o"]
    bk, bvb, bo = w["b2k"], w["b2v"], w["b2o"]
    pW, pb = extras["pw"], extras["pb"]

    KT = sbuf.tile([128, N], bf16, tag="pKT")
    for c in kch_order:
        ps = psP.tile([128, QCH], fp32, tag="proj")
        nc.tensor.matmul(ps[:], lhsT=Wk[:], rhs=XTfull[:, c * QCH:(c + 1) * QCH],
                         start=True, stop=True)
        nc.vector.tensor_tensor(out=KT[:, c * QCH:(c + 1) * QCH], in0=ps[:],
                                in1=_bcast(bk[:, 0:1], QCH), op=ALU.add)
    V = sbuf.tile([128, N], bf16, tag="pV")
    for t in kt_order:
        ps = psP.tile([128, QCH], fp32, tag="proj")
        nc.tensor.matmul(ps[:, 0:128], lhsT=XTfull[:, t * 128:(t + 1) * 128],
                         rhs=Wv[:], start=True, stop=True)
        nc.vector.tensor_tensor(out=V[:, t * 128:(t + 1) * 128], in0=ps[:, 0:128],
                                in1=bvb[:], op=ALU.add)

    # logits L[key, h] via block-diagonal Qhat (cross-head terms hit zeros);
    # column 4*idx holds the idx-th processed key tile.
    Lp_t = psF.tile([128, QCH], fp32, tag="F")
    Lp = Lp_t[:, 0:4 * NKT]
    for idx, t in enumerate(kt_order):
        nc.tensor.matmul(Lp[:, 4 * idx:4 * idx + 4],
                         lhsT=KT[:, t * 128:(t + 1) * 128], rhs=Qhat[:],
                         start=True, stop=True)
    Ap = sbuf.tile([128, 4 * NKT], bf16, tag="Ap")
    half = 2 * NKT
    nc.scalar.activation(Ap[:, 0:half], Lp[:, 0:half], SIG, scale=float(SCALE))
    nc.scalar.activation(Ap[:, half:2 * half], Lp[:, half:2 * half], SIG,
                         scale=float(SCALE))

    # O[1, d] = sum_m A_h(d)[m] V[m, d]
    Ops_t = psF.tile([128, QCH], fp32, tag="F")
    Ops = Ops_t[0:1, 0:128]
    for h in range(H):
        for idx, t in enumerate(kt_order):
            nc.tensor.matmul(
                Ops[0:1, 32 * h:32 * h + 32],
                lhsT=Ap[:, 4 * idx + h:4 * idx + h + 1],
                rhs=V[:, 128 * t + 32 * h:128 * t + 32 * h + 32],
                start=(idx == 0), stop=(idx == NKT - 1), skip_group_check=True)
    Ob = sbuf.tile([1, 128], bf16, tag="Ob")
    nc.vector.tensor_copy(Ob[:], Ops[:])
    # transpose O -> O^T via K=1 matmul with ones
    ones = sbuf.tile([1, 1], bf16, tag="ones")
    nc.vector.memset(ones[:], 1.0)
    OpTps_t = psF.tile([128, QCH], fp32, tag="F")
    OpTps = OpTps_t[:, 0:1]
    nc.tensor.matmul(OpTps[:], lhsT=Ob[:], rhs=ones[:], start=True, stop=True)
    OpTf = sbuf.tile([128, 1], fp32, tag="OpTf")
    OpTb = sbuf.tile([128, 1], bf16, tag="OpTb")
    nc.vector.tensor_tensor(out=OpTf[:], in0=OpTps[:], in1=QpTf[:], op=ALU.add)
    nc.vector.tensor_tensor(out=OpTb[:], in0=OpTps[:], in1=QpTf[:], op=ALU.add)
    # fc_o + relu + residual
    FC2_t = psF.tile([128, QCH], fp32, tag="F")
    FC2 = FC2_t[:, 0:1]
    nc.tensor.matmul(FC2[:], lhsT=Wo[:], rhs=OpTb[:], start=True, stop=True)
    R2 = sbuf.tile([128, 1], fp32, tag="R2")
    nc.vector.tensor_scalar(out=R2[:], in0=FC2[:], scalar1=bo[:, 0:1],
                            scalar2=0.0, op0=ALU.add, op1=ALU.max)
    XpTb = sbuf.tile([128, 1], bf16, tag="XpTb")
    nc.vector.tensor_tensor(out=XpTb[:], in0=OpTf[:], in1=R2[:], op=ALU.add)
    # final [1,256] = P @ pW + pb
    OUTps_t = psF.tile([128, QCH], fp32, tag="F")
    OUTps = OUTps_t[0:1, 0:DOUT]
    nc.tensor.matmul(OUTps[:], lhsT=XpTb[:], rhs=pW[:], start=True, stop=True)
    out_sb = sbuf.tile([1, DOUT], fp32, tag="out_sb")
    nc.vector.tensor_tensor(out=out_sb[:], in0=OUTps[:], in1=pb[:], op=ALU.add)
    if DEBUG_TAPS:
        for nm, t in (("dpKT", KT), ("dpV", V), ("dpAp", Ap), ("dpOpTf", OpTf),
                      ("dpXpTb", XpTb)):
            dd = nc.declare_dram_parameter(nm, list(t[:].shape), fp32,
                                           isOutput=True)
            nc.gpsimd.dma_start(out=dd[:], in_=t[:])
    return out_sb


def build_program():
    nc = bass.Bass(num_devices=8)
    xt = nc.declare_dram_parameter("xt", [128, N], bf16, isOutput=False)
    xtq = nc.declare_dram_parameter("xtq", [128, NQ], bf16, isOutput=False)
    out_d = nc.declare_dram_parameter("out", [1, DOUT], fp32, isOutput=True)

    wshapes = {}
    for i in range(3):
        for k in ("q", "k", "v", "o"):
            wshapes[f"w{i}{k}"] = ([128, 128], bf16)
        wshapes[f"b{i}q"] = ([128, 1], fp32)
        wshapes[f"b{i}k"] = ([128, 1], fp32)
        wshapes[f"b{i}v"] = ([128, 128], fp32)  # pre-broadcast across partitions
        wshapes[f"b{i}o"] = ([128, 1], fp32)
    eshapes = {
        "st": ([128, 1], bf16),
        "hmask": ([128, H], bf16),
        "pw": ([128, DOUT], bf16),
        "pb": ([1, DOUT], fp32),
    }

    with tile.TileContext(nc) as tc:
        with (
            tc.tile_pool(name="sbuf", bufs=1) as sbuf,
            tc.tile_pool(name="sbufA", bufs=3) as sbufA,
            tc.tile_pool(name="psL", bufs=2, space="PSUM") as psL,
            tc.tile_pool(name="psO", bufs=1, space="PSUM") as psO,
            tc.tile_pool(name="psP", bufs=2, space="PSUM") as psP,
            tc.tile_pool(name="psF", bufs=1, space="PSUM") as psF,
            tc.tile_pool(name="dram", bufs=1, space="DRAM") as dram,
        ):
            pools = {"sbuf": sbuf, "sbufA": sbufA, "psL": psL, "psO": psO,
                     "psP": psP, "psF": psF, "dram": dram}

            w = _load_weights(nc, sbuf, wshapes)
            extras = _load_weights(nc, sbuf, eshapes)
            # warm the ACT sigmoid table off the critical path
            warm = sbuf.tile([1, 1], fp32, tag="warm")
            nc.scalar.activation(warm[:], extras["pb"][0:1, 0:1], SIG)

            XT0 = sbuf.tile([128, N], bf16, tag="XT0")
            for c in range(4):
                nc.sync.dma_start(out=XT0[:, c * QCH:(c + 1) * QCH],
                                  in_=xt[:, c * QCH:(c + 1) * QCH])
            XTq0 = sbuf.tile([128, NQ], bf16, tag="XTq0")
            for c in range(2):
                nc.sync.dma_start(out=XTq0[:, c * QCH:(c + 1) * QCH],
                                  in_=xtq[:, c * QCH:(c + 1) * QCH])

            Xh0, XT1 = _sab(nc, pools, XT0, XTq0, w, 0, "s0",
                            [0, 1, 2, 3], list(range(NKT)), emit_ag=True)
            QpTf, Qhat = _pma_q(nc, pools, w, extras)
            Xh1, XT2 = _sab(nc, pools, XT1, Xh0, w, 1, "s1",
                            KCH_ORDER_AG, KT_ORDER_AG, emit_ag=True)
            out_sb = _pma(nc, pools, XT2, w, extras, QpTf, Qhat,
                          KCH_ORDER_AG, KT_ORDER_AG)
            nc.sync.dma_start(out=out_d[:], in_=out_sb[:])

    _fix_excess_waits(nc)
    return nc


_CACHE = {}


def _inputs_for_core(inputs, c):
    b, hf = c // 2, c % 2
    X = np.asarray(inputs["X"], dtype=np.float32)
    XT = np.ascontiguousarray(X[b].T).astype(ml_dtypes.bfloat16)
    m = {
        "xt": XT,
        "xtq": np.ascontiguousarray(XT[:, hf * NQ:(hf + 1) * NQ]),
        "st": np.ascontiguousarray(
            np.asarray(inputs["S"], np.float32).reshape(D, 1)
        ).astype(ml_dtypes.bfloat16),
        "hmask": (np.arange(128)[:, None] // 32 == np.arange(H)[None, :]
                  ).astype(ml_dtypes.bfloat16),
        "pw": np.ascontiguousarray(
            np.asarray(inputs["pW"], np.float32)).astype(ml_dtypes.bfloat16),
        "pb": np.asarray(inputs["pb"], np.float32).reshape(1, DOUT),
    }
    for i in range(3):
        for k in ("q", "k", "v", "o"):
            m[f"w{i}{k}"] = np.ascontiguousarray(
                np.asarray(inputs[f"m{i}_W{k}"], np.float32)
            ).astype(ml_dtypes.bfloat16)
        m[f"b{i}q"] = np.asarray(inputs[f"m{i}_bq"], np.float32).reshape(128, 1)
        m[f"b{i}k"] = np.asarray(inputs[f"m{i}_bk"], np.float32).reshape(128, 1)
        m[f"b{i}v"] = np.tile(
            np.asarray(inputs[f"m{i}_bv"], np.float32)[None, :], (128, 1))
        m[f"b{i}o"] = np.asarray(inputs[f"m{i}_bo"], np.float32).reshape(128, 1)
    return m


def kernel(**inputs) -> np.ndarray:
    if "nc" not in _CACHE:
        _CACHE["nc"] = build_program()
    nc = _CACHE["nc"]
    in_maps = [_inputs_for_core(inputs, c) for c in range(8)]
    res = run_bass_kernel_spmd(nc, in_maps, list(range(8)))
    out = np.stack([res.results[2 * b]["out"] for b in range(B)], axis=0)
    return out.astype(np.float32)  # [B, 1, DOUT]


# revision 14
# speedup vs baseline: 1.1282x; 1.0664x over previous
"""Set-Transformer encoder (2x SAB sigmoid-attention + PMA) on 8 TRN2 cores.

Sharding: core c handles batch b=c//2, query-half hf=c%2 (1024 of 2048 rows).
All data flows feature-major ([D=128 partitions, tokens]); the host supplies
X pre-transposed and pre-cast to bf16.  Between SAB layers each core pair
AllGathers its half of the layer output in two query-chunks, launched as
soon as each chunk is ready so the exchange hides under the remaining
attention work; the next layer processes the keys covered by the first
chunk before the second arrives (attention is permutation-invariant over
keys).  The PMA + final projection are computed redundantly by both cores
of a pair.

Matmul operands are bf16 (1 cycle/row on PE); accumulation and the residual
spine stay fp32.  The per-head (dh=32) QK matmuls use 32x32 tile_position
packing (8 concurrent tiles per 128-key group); AV contracts the full 128
keys with col-banded (M=32) matmuls accumulating O^T in place.
"""
import numpy as np
import ml_dtypes

import concourse.bass as bass
import concourse.tile as tile
from concourse import mybir
from concourse.bass_utils import run_bass_kernel_spmd

B, N, D, H, DH, DOUT = 4, 2048, 128, 4, 32, 256
NQ = N // 2          # queries per core
QCH = 512            # query chunk (matmul moving-dim)
NKT = N // 128       # 16 key tiles
SCALE = 1.0 / np.sqrt(np.float32(D))  # 1/sqrt(128) logit scale

fp32 = mybir.dt.float32
bf16 = mybir.dt.bfloat16
ALU = mybir.AluOpType
SIG = mybir.ActivationFunctionType.Sigmoid
PAIRS = [[0, 1], [2, 3], [4, 5], [6, 7]]
DEBUG_TAPS = False

# key-tile processing order when keys arrive via 2-chunk AllGather:
# AG chunk a carries each core's queries [0:512) -> global keys
# [0:512) u [1024:1536) = key tiles 0-3 and 8-11.
KT_ORDER_AG = [0, 1, 2, 3, 8, 9, 10, 11, 4, 5, 6, 7, 12, 13, 14, 15]
KCH_ORDER_AG = [0, 2, 1, 3]          # 512-col projection chunk order


def _fix_excess_waits(nc):
    """walrus accepts very few sync waits per instruction; hoist excess
    waits onto preceding same-engine NOPs (same stream => same semantics)."""
    for f in nc.m.functions:
        for bb in f.blocks:
            new_list = []
            for ins in bb.instructions:
                si = ins.sync_info
                cap = 2 if isinstance(ins, mybir.InstEventSemaphore) else 1
                if si is not None and len(si.on_wait) > cap:
                    waits = list(si.on_wait)
                    excess, kept = waits[:-cap], waits[-cap:]
                    for j, w in enumerate(excess):
                        nop = mybir.InstNoOp(
                            name=f"{ins.name}-presync{j}", ins=[], outs=[]
                        )
                        nop.engine = ins.engine
                        nop.sync_info = mybir.SyncInfo(on_wait=[w], on_update=[])
                        nc.register_instruction(nop)
                        new_list.append(nop)
                    ins.sync_info = mybir.SyncInfo(
                        on_wait=kept, on_update=list(si.on_update)
                    )
                new_list.append(ins)
            bb.instructions = new_list


def _bcast(ap, n):
    return ap.to_broadcast([ap.shape[0], n])


def _load_weights(nc, sbuf, shapes):
    tiles = {}
    for key, (shape, dt) in shapes.items():
        p = nc.declare_dram_parameter(key, shape, dt if dt == bf16 else fp32,
                                      isOutput=False)
        t = sbuf.tile(shape, dt, tag=f"in_{key}")
        nc.gpsimd.dma_start(out=t[:], in_=p[:])
        tiles[key] = t
    return tiles


def _proj_q(nc, pools, XTq, w, i, tagp):
    sbuf, psP = pools["sbuf"], pools["psP"]
    Wq, bq = w[f"w{i}q"], w[f"b{i}q"]
    QTf = sbuf.tile([128, NQ], fp32, tag=f"{tagp}QTf")
    QTb = sbuf.tile([128, NQ], bf16, tag=f"{tagp}QTb")
    for c in range(2):
        ps = psP.tile([128, QCH], fp32, tag="proj")
        nc.tensor.matmul(ps[:], lhsT=Wq[:], rhs=XTq[:, c * QCH:(c + 1) * QCH],
                         start=True, stop=True)
        nc.vector.tensor_tensor(
            out=QTf[:, c * QCH:(c + 1) * QCH], in0=ps[:],
            in1=_bcast(bq[:, 0:1], QCH), op=ALU.add)
        nc.vector.tensor_tensor(
            out=QTb[:, c * QCH:(c + 1) * QCH], in0=ps[:],
            in1=_bcast(bq[:, 0:1], QCH), op=ALU.add)
    return QTf, QTb


def _proj_kv_wave(nc, pools, XTfull, w, i, KT, V, kchs, kts):
    """K^T chunks + V tiles for one wave of arrived keys."""
    psP = pools["psP"]
    Wk, Wv = w[f"w{i}k"], w[f"w{i}v"]
    bk, bvb = w[f"b{i}k"], w[f"b{i}v"]
    for c in kchs:
        ps = psP.tile([128, QCH], fp32, tag="proj")
        nc.tensor.matmul(ps[:], lhsT=Wk[:], rhs=XTfull[:, c * QCH:(c + 1) * QCH],
                         start=True, stop=True)
        nc.vector.tensor_tensor(
            out=KT[:, c * QCH:(c + 1) * QCH], in0=ps[:],
            in1=_bcast(bk[:, 0:1], QCH), op=ALU.add)
    for t in kts:
        ps = psP.tile([128, QCH], fp32, tag="proj")
        nc.tensor.matmul(ps[:, 0:128], lhsT=XTfull[:, t * 128:(t + 1) * 128],
                         rhs=Wv[:], start=True, stop=True)
        nc.vector.tensor_tensor(out=V[:, t * 128:(t + 1) * 128], in0=ps[:, 0:128],
                                in1=bvb[:], op=ALU.add)


def _sab(nc, pools, XTfull, XTq, w, i, tagp, waves, emit_ag):
    """One SAB layer; returns (XhT_half, XTnext or None).

    waves: list of (kch_list, kt_list) -- keys grouped by arrival order."""
    sbuf, sbufA, psL, psO, psF, dram = (
        pools["sbuf"], pools["sbufA"], pools["psL"], pools["psO"], pools["psF"],
        pools["dram"],
    )
    Wo, bo = w[f"w{i}o"], w[f"b{i}o"]
    kt_order = [t for _, kts in waves for t in kts]
    QTf, QTb = _proj_q(nc, pools, XTq, w, i, tagp)
    KT = sbuf.tile([128, N], bf16, tag=f"{tagp}KT")
    V = sbuf.tile([128, N], bf16, tag=f"{tagp}V")

    XhT = sbuf.tile([128, NQ], bf16, tag=f"{tagp}XhT")
    OTf = sbuf.tile([128, NQ], fp32, tag=f"{tagp}OTf")
    OTb = sbuf.tile([128, NQ], bf16, tag=f"{tagp}OTb")
    XTnext = None
    if emit_ag:
        XTnext = sbuf.tile([128, N], bf16, tag=f"{tagp}XTn")

    for qc in range(2):
        qs = qc * QCH
        OTps = psO.tile([128, QCH], fp32, tag="OT")
        groups = [(kt, hp) for kt in kt_order for hp in range(2)]
        Ltiles = {}
        # wave w's projections are emitted just before its first group
        # (first qc pass only); the scheduler starts them as keys arrive.
        proj_at = {}
        if qc == 0:
            gidx = 0
            for kchs, kts in waves:
                proj_at[gidx] = (kchs, kts)
                gidx += 2 * len(kts)

        def emit_qk(g):
            kt, hp = groups[g]
            L = psL.tile([128, 1024], fp32, tag="L")
            Ltiles[g] = L
            for h in (2 * hp, 2 * hp + 1):
                for j in range(4):
                    nc.tensor.matmul(
                        out=L[32 * j:32 * j + 32,
                              QCH * (h - 2 * hp):QCH * (h - 2 * hp) + QCH],
                        lhsT=KT[32 * h:32 * h + 32,
                                128 * kt + 32 * j:128 * kt + 32 * j + 32],
                        rhs=QTb[32 * h:32 * h + 32, qs:qs + QCH],
                        start=True, stop=True,
                        tile_position=(32 * h, 32 * j))

        ng = len(groups)
        if 0 in proj_at:
            _proj_kv_wave(nc, pools, XTfull, w, i, KT, V, *proj_at[0])
        emit_qk(0)
        for g in range(ng):
            kt, hp = groups[g]
            if g + 1 in proj_at:
                _proj_kv_wave(nc, pools, XTfull, w, i, KT, V, *proj_at[g + 1])
            if g + 1 < ng:
                emit_qk(g + 1)
            A = sbufA.tile([128, 1024], bf16, tag="A")
            nc.scalar.activation(A[:], Ltiles.pop(g)[:], SIG, scale=float(SCALE))
            # AV: A holds the full 128 keys of tile kt on partitions;
            # contract K=128 with one col-banded matmul per head.
            for h in (2 * hp, 2 * hp + 1):
                nc.tensor.matmul(
                    out=OTps[32 * h:32 * h + 32, 0:QCH],
                    lhsT=V[:, 128 * kt + 32 * h:128 * kt + 32 * h + 32],
                    rhs=A[:, QCH * (h - 2 * hp):QCH * (h - 2 * hp) + QCH],
                    start=(g // 2 == 0), stop=(g // 2 == NKT - 1),
                    tile_position=(0, 32 * h),
                    skip_group_check=True)

        # O = Qp + A@V ; Xh = O + relu(O @ Wo + bo)
        nc.vector.tensor_tensor(out=OTf[:, qs:qs + QCH], in0=OTps[:],
                                in1=QTf[:, qs:qs + QCH], op=ALU.add)
        nc.vector.tensor_tensor(out=OTb[:, qs:qs + QCH], in0=OTps[:],
                                in1=QTf[:, qs:qs + QCH], op=ALU.add)
        FC = psF.tile([128, QCH], fp32, tag="F")
        nc.tensor.matmul(FC[:], lhsT=Wo[:], rhs=OTb[:, qs:qs + QCH],
                         start=True, stop=True)
        R = sbuf.tile([128, QCH], fp32, tag="R")
        nc.vector.tensor_scalar(out=R[:], in0=FC[:], scalar1=bo[:, 0:1],
                                scalar2=0.0, op0=ALU.add, op1=ALU.max)
        nc.vector.tensor_tensor(out=XhT[:, qs:qs + QCH], in0=OTf[:, qs:qs + QCH],
                                in1=R[:], op=ALU.add)

        if emit_ag:
            # exchange this query chunk with the pair core right away
            cc_in = dram.tile([128, QCH], bf16, tag=f"{tagp}cci{qc}")
            nc.sync.dma_start(out=cc_in[:], in_=XhT[:, qs:qs + QCH])
            cc_out = dram.tile([256, QCH], bf16, tag=f"{tagp}cco{qc}")
            nc.gpsimd.collective_compute(
                "AllGather", ALU.bypass, replica_groups=PAIRS,
                ins=[cc_in[:]], outs=[cc_out[:]])
            # global columns: rank0 rows -> [qs:qs+512), rank1 -> [1024+qs:...)
            nc.sync.dma_start(out=XTnext[:, qs:qs + QCH], in_=cc_out[0:128, :])
            nc.sync.dma_start(out=XTnext[:, NQ + qs:NQ + qs + QCH],
                              in_=cc_out[128:256, :])

    if DEBUG_TAPS:
        for nm, t in ((f"d{i}KT", KT), (f"d{i}QTb", QTb), (f"d{i}QTf", QTf),
                      (f"d{i}V", V), (f"d{i}OTf", OTf), (f"d{i}XhT", XhT)):
            dd = nc.declare_dram_parameter(nm, list(t[:].shape), fp32,
                                           isOutput=True)
            nc.gpsimd.dma_start(out=dd[:], in_=t[:])
    return XhT, XTnext


def _pma_q(nc, pools, w, extras):
    """PMA seed query (depends only on S + mab2 weights) - emitted early."""
    sbuf, psP = pools["sbuf"], pools["psP"]
    Wq, bq = w["w2q"], w["b2q"]
    ST, hmask = extras["st"], extras["hmask"]
    psq = psP.tile([128, QCH], fp32, tag="proj")
    nc.tensor.matmul(psq[:, 0:1], lhsT=Wq[:], rhs=ST[:, 0:1], start=True,
                     stop=True)
    QpTf = sbuf.tile([128, 1], fp32, tag="QpTf")
    QpTb = sbuf.tile([128, 1], bf16, tag="QpTb")
    nc.vector.tensor_tensor(out=QpTf[:], in0=psq[:, 0:1], in1=bq[:, 0:1],
                            op=ALU.add)
    nc.vector.tensor_copy(QpTb[:], QpTf[:])
    # Block-diagonal Qhat[d, h] = Qp^T[d] * (d//32 == h)
    Qhat = sbuf.tile([128, H], bf16, tag="Qhat")
    nc.vector.tensor_tensor(out=Qhat[:], in0=_bcast(QpTb[:, 0:1], H),
                            in1=hmask[:], op=ALU.mult)
    return QpTf, Qhat


def _pma(nc, pools, XTfull, w, extras, QpTf, Qhat, kch_order, kt_order):
    """PMA (1 seed) + final projection -> out [1, 256] fp32 in SBUF."""
    sbuf, psP, psF = pools["sbuf"], pools["psP"], pools["psF"]
    Wk, Wv, Wo = w["w2k"], w["w2v"], w["w2o"]
    bk, bvb, bo = w["b2k"], w["b2v"], w["b2o"]
    pW, pb = extras["pw"], extras["pb"]

    KT = sbuf.tile([128, N], bf16, tag="pKT")
    for c in kch_order:
        ps = psP.tile([128, QCH], fp32, tag="proj")
        nc.tensor.matmul(ps[:], lhsT=Wk[:], rhs=XTfull[:, c * QCH:(c + 1) * QCH],
                         start=True, stop=True)
        nc.vector.tensor_tensor(out=KT[:, c * QCH:(c + 1) * QCH], in0=ps[:],
                                in1=_bcast(bk[:, 0:1], QCH), op=ALU.add)
    V = sbuf.tile([128, N], bf16, tag="pV")
    for t in kt_order:
        ps = psP.tile([128, QCH], fp32, tag="proj")
        nc.tensor.matmul(ps[:, 0:128], lhsT=XTfull[:, t * 128:(t + 1) * 128],
                         rhs=Wv[:], start=True, stop=True)
        nc.vector.tensor_tensor(out=V[:, t * 128:(t + 1) * 128], in0=ps[:, 0:128],
                                in1=bvb[:], op=ALU.add)

    # logits L[key, h] via block-diagonal Qhat (cross-head terms hit zeros);
    # column 4*idx holds the idx-th processed key tile.
    Lp_t = psF.tile([128, QCH], fp32, tag="F")
    Lp = Lp_t[:, 0:4 * NKT]
    for idx, t in enumerate(kt_order):
        nc.tensor.matmul(Lp[:, 4 * idx:4 * idx + 4],
                         lhsT=KT[:, t * 128:(t + 1) * 128], rhs=Qhat[:],
                         start=True, stop=True)
    Ap = sbuf.tile([128, 4 * NKT], bf16, tag="Ap")
    half = 2 * NKT
    nc.scalar.activation(Ap[:, 0:half], Lp[:, 0:half], SIG, scale=float(SCALE))
    nc.scalar.activation(Ap[:, half:2 * half], Lp[:, half:2 * half], SIG,
                         scale=float(SCALE))

    # O[1, d] = sum_m A_h(d)[m] V[m, d]
    Ops_t = psF.tile([128, QCH], fp32, tag="F")
    Ops = Ops_t[0:1, 0:128]
    for h in range(H):
        for idx, t in enumerate(kt_order):
            nc.tensor.matmul(
                Ops[0:1, 32 * h:32 * h + 32],
                lhsT=Ap[:, 4 * idx + h:4 * idx + h + 1],
                rhs=V[:, 128 * t + 32 * h:128 * t + 32 * h + 32],
                start=(idx == 0), stop=(idx == NKT - 1), skip_group_check=True)
    Ob = sbuf.tile([1, 128], bf16, tag="Ob")
    nc.vector.tensor_copy(Ob[:], Ops[:])
    # transpose O -> O^T via K=1 matmul with ones
    ones = sbuf.tile([1, 1], bf16, tag="ones")
    nc.vector.memset(ones[:], 1.0)
    OpTps_t = psF.tile([128, QCH], fp32, tag="F")
    OpTps = OpTps_t[:, 0:1]
    nc.tensor.matmul(OpTps[:], lhsT=Ob[:], rhs=ones[:], start=True, stop=True)
    OpTf = sbuf.tile([128, 1], fp32, tag="OpTf")
    OpTb = sbuf.tile([128, 1], bf16, tag="OpTb")
    nc.vector.tensor_tensor(out=OpTf[:], in0=OpTps[:], in1=QpTf[:], op=ALU.add)
    nc.vector.tensor_tensor(out=OpTb[:], in0=OpTps[:], in1=QpTf[:], op=ALU.add)
    # fc_o + relu + residual
    FC2_t = psF.tile([128, QCH], fp32, tag="F")
    FC2 = FC2_t[:, 0:1]
    nc.tensor.matmul(FC2[:], lhsT=Wo[:], rhs=OpTb[:], start=True, stop=True)
    R2 = sbuf.tile([128, 1], fp32, tag="R2")
    nc.vector.tensor_scalar(out=R2[:], in0=FC2[:], scalar1=bo[:, 0:1],
                            scalar2=0.0, op0=ALU.add, op1=ALU.max)
    XpTb = sbuf.tile([128, 1], bf16, tag="XpTb")
    nc.vector.tensor_tensor(out=XpTb[:], in0=OpTf[:], in1=R2[:], op=ALU.add)
    # final [1,256] = P @ pW + pb
    OUTps_t = psF.tile([128, QCH], fp32, tag="F")
    OUTps = OUTps_t[0:1, 0:DOUT]
    nc.tensor.matmul(OUTps[:], lhsT=XpTb[:], rhs=pW[:], start=True, stop=True)
    out_sb = sbuf.tile([1, DOUT], fp32, tag="out_sb")
    nc.vector.tensor_tensor(out=out_sb[:], in0=OUTps[:], in1=pb[:], op=ALU.add)
    if DEBUG_TAPS:
        for nm, t in (("dpKT", KT), ("dpV", V), ("dpAp", Ap), ("dpOpTf", OpTf),
                      ("dpXpTb", XpTb)):
            dd = nc.declare_dram_parameter(nm, list(t[:].shape), fp32,
                                           isOutput=True)
            nc.gpsimd.dma_start(out=dd[:], in_=t[:])
    return out_sb


def build_program():
    nc = bass.Bass(num_devices=8)
    xt = nc.declare_dram_parameter("xt", [128, N], bf16, isOutput=False)
    xtq = nc.declare_dram_parameter("xtq", [128, NQ], bf16, isOutput=False)
    out_d = nc.declare_dram_parameter("out", [1, DOUT], fp32, isOutput=True)

    wshapes = {}
    for i in range(3):
        for k in ("q", "k", "v", "o"):
            wshapes[f"w{i}{k}"] = ([128, 128], bf16)
        wshapes[f"b{i}q"] = ([128, 1], fp32)
        wshapes[f"b{i}k"] = ([128, 1], fp32)
        wshapes[f"b{i}v"] = ([128, 128], fp32)  # pre-broadcast across partitions
        wshapes[f"b{i}o"] = ([128, 1], fp32)
    eshapes = {
        "st": ([128, 1], bf16),
        "hmask": ([128, H], bf16),
        "pw": ([128, DOUT], bf16),
        "pb": ([1, DOUT], fp32),
    }

    with tile.TileContext(nc) as tc:
        with (
            tc.tile_pool(name="sbuf", bufs=1) as sbuf,
            tc.tile_pool(name="sbufA", bufs=3) as sbufA,
            tc.tile_pool(name="psL", bufs=2, space="PSUM") as psL,
            tc.tile_pool(name="psO", bufs=1, space="PSUM") as psO,
            tc.tile_pool(name="psP", bufs=2, space="PSUM") as psP,
            tc.tile_pool(name="psF", bufs=1, space="PSUM") as psF,
            tc.tile_pool(name="dram", bufs=1, space="DRAM") as dram,
        ):
            pools = {"sbuf": sbuf, "sbufA": sbufA, "psL": psL, "psO": psO,
                     "psP": psP, "psF": psF, "dram": dram}

            # inputs: xt/xtq chunks on the HW-DGE queue, weights on SW-DGE
            XT0 = sbuf.tile([128, N], bf16, tag="XT0")
            for c in range(4):
                nc.sync.dma_start(out=XT0[:, c * QCH:(c + 1) * QCH],
                                  in_=xt[:, c * QCH:(c + 1) * QCH])
            XTq0 = sbuf.tile([128, NQ], bf16, tag="XTq0")
            for c in range(2):
                nc.sync.dma_start(out=XTq0[:, c * QCH:(c + 1) * QCH],
                                  in_=xtq[:, c * QCH:(c + 1) * QCH])
            w0 = {k: v for k, v in wshapes.items() if "0" in k}
            wrest = {k: v for k, v in wshapes.items() if "0" not in k}
            w = _load_weights(nc, sbuf, w0)
            w.update(_load_weights(nc, sbuf, wrest))
            extras = _load_weights(nc, sbuf, eshapes)
            # warm the ACT sigmoid table off the critical path
            warm = sbuf.tile([1, 1], fp32, tag="warm")
            nc.scalar.activation(warm[:], extras["pb"][0:1, 0:1], SIG)

            WAVES0 = [([0, 1], [0, 1, 2, 3, 4, 5, 6, 7]),
                      ([2, 3], [8, 9, 10, 11, 12, 13, 14, 15])]
            WAVES_AG = [([0, 2], [0, 1, 2, 3, 8, 9, 10, 11]),
                        ([1, 3], [4, 5, 6, 7, 12, 13, 14, 15])]
            Xh0, XT1 = _sab(nc, pools, XT0, XTq0, w, 0, "s0", WAVES0,
                            emit_ag=True)
            QpTf, Qhat = _pma_q(nc, pools, w, extras)
            Xh1, XT2 = _sab(nc, pools, XT1, Xh0, w, 1, "s1", WAVES_AG,
                            emit_ag=True)
            out_sb = _pma(nc, pools, XT2, w, extras, QpTf, Qhat,
                          KCH_ORDER_AG, KT_ORDER_AG)
            nc.sync.dma_start(out=out_d[:], in_=out_sb[:])

    _fix_excess_waits(nc)
    return nc


_CACHE = {}


def _inputs_for_core(inputs, c):
    b, hf = c // 2, c % 2
    X = np.asarray(inputs["X"], dtype=np.float32)
    XT = np.ascontiguousarray(X[b].T).astype(ml_dtypes.bfloat16)
    m = {
        "xt": XT,
        "xtq": np.ascontiguousarray(XT[:, hf * NQ:(hf + 1) * NQ]),
        "st": np.ascontiguousarray(
            np.asarray(inputs["S"], np.float32).reshape(D, 1)
        ).astype(ml_dtypes.bfloat16),
        "hmask": (np.arange(128)[:, None] // 32 == np.arange(H)[None, :]
                  ).astype(ml_dtypes.bfloat16),
        "pw": np.ascontiguousarray(
            np.asarray(inputs["pW"], np.float32)).astype(ml_dtypes.bfloat16),
        "pb": np.asarray(inputs["pb"], np.float32).reshape(1, DOUT),
    }
    for i in range(3):
        for k in ("q", "k", "v", "o"):
            m[f"w{i}{k}"] = np.ascontiguousarray(
                np.asarray(inputs[f"m{i}_W{k}"], np.float32)
            ).astype(ml_dtypes.bfloat16)
        m[f"b{i}q"] = np.asarray(inputs[f"m{i}_bq"], np.float32).reshape(128, 1)
        m[f"b{i}k"] = np.asarray(inputs[f"m{i}_bk"], np.float32).reshape(128, 1)
        m[f"b{i}v"] = np.tile(
            np.asarray(inputs[f"m{i}_bv"], np.float32)[None, :], (128, 1))
        m[f"b{i}o"] = np.asarray(inputs[f"m{i}_bo"], np.float32).reshape(128, 1)
    return m


def kernel(**inputs) -> np.ndarray:
    if "nc" not in _CACHE:
        _CACHE["nc"] = build_program()
    nc = _CACHE["nc"]
    in_maps = [_inputs_for_core(inputs, c) for c in range(8)]
    res = run_bass_kernel_spmd(nc, in_maps, list(range(8)))
    out = np.stack([res.results[2 * b]["out"] for b in range(B)], axis=0)
    return out.astype(np.float32)  # [B, 1, DOUT]
